# revision 1
# baseline (speedup 1.0000x reference)
"""Deformable Conv1d (B=4, C=256, L=8192, K=3, DG=4) on 8 Trainium2 cores.

Sharding: core = (sample b = core//2, L-half h = core%2); each core computes
out[b, :, h*4096:(h+1)*4096] from a haloed window of x[b].

Per-core pipeline:
  conv (PE, bf16): offset/mask convs as shifted-rhs matmuls + an iota matmul
    so PSUM holds p = off + l + (k-1) + HALO after the per-row drain bias.
  small chain (DVE, packed [96,512]): t = p mod 1, a0=(1-t)*m, a1=t*m,
    idx = int16(p - t) clamped.
  dma_gather (transpose=True) from per-dgroup transposed pair tables:
    row e = [x[c, e] for c in dgroup] ++ [x[c, e+1] for c]  (256B bf16 rows).
  A-broadcast (PE selector matmul): replicates a0 to partitions 0-63 and a1
    to 64-127, reading the a-plane through a sigma_c-permuted rhs AP.
  modulate (DVE): M = G * A;  S[64*(dk%2)+...] = M[0:64] + M[64:128].
  main matmul (PE): out = W2 @ S + bias with sigma_c-unpermuting rhs AP.
"""
import sys
sys.path.insert(0, '/opt/trn_rl_repo')
from contextlib import ExitStack
import numpy as np
import ml_dtypes

import concourse.bass as bass
import concourse.tile as tile
from concourse import bacc, mybir

dt = mybir.dt
bf16 = ml_dtypes.bfloat16

B, C, L = 4, 256, 8192
N_CORES = 8
LH = L // 2
HALO = 17
W = LH + 2 * HALO          # 4130 window positions
WROWS = 33 * 128           # 4224 padded rows in pair tables
NCHUNK = 8
CH = 512
NBATCH = 4                 # 2-chunk modulate batches
BCH = 2 * CH
AF = mybir.ActivationFunctionType
ALU = mybir.AluOpType


def build_program(n_reps=1):
    nc = bacc.Bacc("TRN2", target_bir_lowering=False, debug=False,
                   enable_asserts=True, num_devices=N_CORES,
                   num_swdge_queues=2, dynamic_dma_scratch_size=24576)

    def din(name, shape, dty):
        return nc.dram_tensor(name, shape, dty, kind="ExternalInput").ap()

    xT = din("xT", (4, 128, WROWS), dt.bfloat16)
    xP = din("xP", (2, 128, W), dt.bfloat16)
    wconv = din("wconv", (2, 3, 128, 44), dt.bfloat16)
    iotas = din("iotas", (2, 128, CH), dt.float32)
    boff = din("boff", (12, 1), dt.float32)
    bmask = din("bmask", (12, 1), dt.float32)
    wmain = din("wmain", (6, 2, 128, 128), dt.bfloat16)
    bmain = din("bmain", (2, 128, 1), dt.float32)
    wsel = din("wsel", (12, 24, 128), dt.bfloat16)
    yout = nc.dram_tensor("y", (2, 128, LH), dt.float32, kind="ExternalOutput").ap()

    with ExitStack() as ctx:
        tc = ctx.enter_context(tile.TileContext(nc))
        cpool = ctx.enter_context(tc.tile_pool(name="const", bufs=1))
        ppool = ctx.enter_context(tc.tile_pool(name="planes", bufs=1))
        gpool = ctx.enter_context(tc.tile_pool(name="g", bufs=2))
        mpool = ctx.enter_context(tc.tile_pool(name="mtp", bufs=2))
        apool = ctx.enter_context(tc.tile_pool(name="a", bufs=1))
        spool = ctx.enter_context(tc.tile_pool(name="s", bufs=1))
        opool = ctx.enter_context(tc.tile_pool(name="o", bufs=2))
        psc = ctx.enter_context(tc.tile_pool(name="psc", bufs=2, space="PSUM"))
        psb = ctx.enter_context(tc.tile_pool(name="psb", bufs=2, space="PSUM"))
        psm = ctx.enter_context(tc.tile_pool(name="psm", bufs=2, space="PSUM"))

        t_xT = [cpool.tile([128, WROWS], dt.bfloat16, tag=f"xT{d}", name=f"xT{d}") for d in range(4)]
        for d in range(4):
            nc.sync.dma_start(t_xT[d][:], xT[d])
        t_xP = [cpool.tile([128, W], dt.bfloat16, tag=f"xP{cb}", name=f"xP{cb}") for cb in range(2)]
        for cb in range(2):
            nc.sync.dma_start(t_xP[cb][:], xP[cb])
        t_wconv = [[cpool.tile([128, 44], dt.bfloat16, tag=f"wc{cb}{k}", name=f"wc{cb}{k}")
                    for k in range(3)] for cb in range(2)]
        for cb in range(2):
            for k in range(3):
                nc.sync.dma_start(t_wconv[cb][k][:], wconv[cb, k])
        t_iot = [cpool.tile([128, CH], dt.float32, tag=f"iot{t}", name=f"iot{t}")
                 for t in range(2)]
        for t in range(2):
            nc.sync.dma_start(t_iot[t][:], iotas[t])
        t_boff = cpool.tile([12, 1], dt.float32, name="boff")
        nc.sync.dma_start(t_boff[:], boff[:])
        t_bmask = cpool.tile([12, 1], dt.float32, name="bmask")
        nc.sync.dma_start(t_bmask[:], bmask[:])
        t_wmain = [[cpool.tile([128, 128], dt.bfloat16, tag=f"wm{kb}{mb}", name=f"wm{kb}{mb}")
                    for mb in range(2)] for kb in range(6)]
        for kb in range(6):
            for mb in range(2):
                nc.sync.dma_start(t_wmain[kb][mb][:], wmain[kb, mb])
        t_bmain = [cpool.tile([128, 1], dt.float32, tag=f"bm{mb}", name=f"bm{mb}") for mb in range(2)]
        for mb in range(2):
            nc.sync.dma_start(t_bmain[mb][:], bmain[mb])
        t_wsel = [cpool.tile([24, 128], dt.bfloat16, tag=f"sel{dk}", name=f"sel{dk}") for dk in range(12)]
        for dk in range(12):
            nc.sync.dma_start(t_wsel[dk][:], wsel[dk])

        for _rep in range(n_reps):
            pk = [ppool.tile([128, CH], dt.float32, tag=f"pk{t}", name=f"pk{t}") for t in range(2)]
            mk = [ppool.tile([128, CH], dt.float32, tag=f"mk{t}", name=f"mk{t}") for t in range(2)]
            ttl = [ppool.tile([128, CH], dt.float32, tag=f"ttl{t}", name=f"ttl{t}") for t in range(2)]
            p0f = [ppool.tile([128, CH], dt.float32, tag=f"p0f{t}", name=f"p0f{t}") for t in range(2)]
            a0p = [ppool.tile([128, CH], dt.bfloat16, tag=f"a0p{t}", name=f"a0p{t}") for t in range(2)]
            a1p = [ppool.tile([128, CH], dt.bfloat16, tag=f"a1p{t}", name=f"a1p{t}") for t in range(2)]
            i16p = [ppool.tile([128, CH], dt.int16, tag=f"i16p{t}", name=f"i16p{t}") for t in range(2)]
            aplane = ppool.tile([24, LH], dt.bfloat16, tag="aplane", name="aplane")
            idxt = [ppool.tile([128, LH // 16], dt.int16, tag=f"idx{dk}", name=f"idx{dk}")
                    for dk in range(12)]

            # ---- conv + drains --------------------------------------------
            for c in range(NCHUNK):
                ps = psc.tile([64, CH], dt.float32, tag="convps", name="convps")
                for cb in range(2):
                    for k in range(3):
                        rhs = t_xP[cb][:, c * CH + HALO - 1 + k:
                                       c * CH + HALO - 1 + k + CH]
                        nc.tensor.matmul(ps[0:44, :], t_wconv[cb][k][:], rhs,
                                         start=(cb == 0 and k == 0),
                                         stop=(cb == 1 and k == 2))
                t, rb = c // 4, 32 * (c % 4)
                nc.scalar.activation(pk[t][rb:rb + 12, :], ps[0:12, :], AF.Identity,
                                     bias=t_boff[:], scale=1.0)
                nc.scalar.activation(mk[t][rb:rb + 12, :], ps[32:44, :], AF.Sigmoid,
                                     bias=t_bmask[:], scale=1.0)

            # ---- small chain (packed [96, CH]) ----------------------------
            for t in range(2):
                nc.vector.tensor_add(pk[t][:], pk[t][:], t_iot[t][:])
                nc.vector.tensor_copy(i16p[t][:], pk[t][:])         # ~round/trunc
                nc.vector.tensor_copy(p0f[t][:], i16p[t][:])
                nc.vector.tensor_sub(ttl[t][:], pk[t][:], p0f[t][:])  # d = p - i
                # floor fixup: if d < 0: i -= 1, d += 1
                nc.vector.tensor_scalar(pk[t][:], ttl[t][:], 0.0, None, ALU.is_lt)
                nc.vector.tensor_sub(p0f[t][:], p0f[t][:], pk[t][:])
                nc.vector.tensor_add(ttl[t][:], ttl[t][:], pk[t][:])  # t in [0,1)
                nc.vector.tensor_mul(ttl[t][:], ttl[t][:], mk[t][:])
                nc.vector.tensor_copy(a1p[t][:], ttl[t][:])
                nc.vector.tensor_sub(a0p[t][:], mk[t][:], ttl[t][:])
                nc.vector.tensor_copy(i16p[t][:], p0f[t][:])
                nc.vector.tensor_scalar_max(i16p[t][:], i16p[t][:], 0)
                nc.vector.tensor_scalar_min(i16p[t][:], i16p[t][:], W - 1)

            # ---- unpack to planes -----------------------------------------
            for c in range(NCHUNK):
                t, rb = c // 4, 32 * (c % 4)
                nc.sync.dma_start(aplane[0:12, c * CH:(c + 1) * CH],
                                  a0p[t][rb:rb + 12, :])
                nc.sync.dma_start(aplane[12:24, c * CH:(c + 1) * CH],
                                  a1p[t][rb:rb + 12, :])

            # ---- idx spread (call-major sigma):
            # idxt[dk][16g+p, 32c+s] = i16p[12*(p//2)+dk, (p%2)*256+32c+s]
            #   => gather call c, output col j=s*16+p holds plane position
            #      l'(j) = 256*(j%16) + 32*c + j//16
            for dk in range(12):
                for g in range(8):
                    nc.sync.dma_start(idxt[dk][16 * g:16 * g + 8, :],
                                      i16p[0][dk:128:32, :])
                    nc.sync.dma_start(idxt[dk][16 * g + 8:16 * g + 16, :],
                                      i16p[1][dk:128:32, :])
            # ---- gather / broadcast / modulate (8 calls of 512) -----------
            shalf = [spool.tile([128, LH], dt.bfloat16, tag=f"s{kb}", name=f"s{kb}")
                     for kb in range(6)]
            ap_sig = aplane[:].rearrange("a (p c s) -> a c s p", p=16, c=8, s=32)
            for c in range(8):
                for dk in range(12):
                    gt = gpool.tile([128, CH], dt.bfloat16, tag=f"g{dk}", name=f"g{dk}")
                    at = apool.tile([128, CH], dt.bfloat16, tag=f"a{dk}", name=f"a{dk}")
                    nc.gpsimd.dma_gather(
                        gt[:].unsqueeze(1),
                        t_xT[dk // 3][:], idxt[dk][:, 32 * c:32 * c + 32],
                        num_idxs=CH, num_idxs_reg=CH, elem_size=128,
                        transpose=True, queue_num=dk % 2,
                        sbuf_tokens_per_rank=128,
                        sbuf_free_dim_per_rank=256)
                    bps = psb.tile([128, CH], dt.float32, tag="bcps", name="bcps")
                    nc.tensor.matmul(bps[:], t_wsel[dk][:], ap_sig[:, c],
                                     start=True, stop=True)
                    nc.scalar.copy(at[:], bps[:])
                    mt = mpool.tile([128, CH], dt.bfloat16, tag=f"mt{dk % 2}", name=f"mt{dk % 2}")
                    nc.vector.tensor_mul(mt[:], gt[:], at[:])
                    v1t = mpool.tile([64, CH], dt.bfloat16, tag=f"v1t{dk % 2}", name=f"v1t{dk % 2}")
                    nc.sync.dma_start(v1t[:], mt[64:128, :])
                    nc.vector.tensor_add(
                        shalf[dk // 2][64 * (dk % 2):64 * (dk % 2) + 64,
                                       c * CH:(c + 1) * CH],
                        mt[0:64, :], v1t[:])
            # ---- main matmuls ---------------------------------------------
            for cn in range(NCHUNK):
                for mb in range(2):
                    mps = psm.tile([128, CH], dt.float32, tag=f"mps{mb}", name=f"mps{mb}")
                    for kb in range(6):
                        rhs = shalf[kb][:].rearrange(
                            "a (c s p) -> a p c s", c=8, s=32, p=16)[:, 2 * cn:2 * cn + 2]
                        nc.tensor.matmul(mps[:], t_wmain[kb][mb][:], rhs,
                                         start=(kb == 0), stop=(kb == 5))
                    ot = opool.tile([128, CH], dt.float32, tag=f"ot{mb}", name=f"ot{mb}")
                    nc.scalar.activation(ot[:], mps[:], AF.Identity,
                                         bias=t_bmain[mb][:], scale=1.0)
                    nc.sync.dma_start(yout[mb, :, cn * CH:(cn + 1) * CH], ot[:])

    nc.compile()
    return nc


# ---------------------------------------------------------------------------

def _prep_core_inputs(x, w_off, b_off, w_mask, b_mask, weight, bias, b, h):
    q0 = h * LH - HALO
    xpad = np.zeros((C, W + 1), np.float32)
    lo, hi = max(0, q0), min(L, q0 + W + 1)
    xpad[:, lo - q0:hi - q0] = x[b][:, lo:hi]
    xpad_bf = xpad.astype(bf16)

    xT = np.zeros((4, 128, WROWS), bf16)
    for d in range(4):
        rows = np.concatenate([xpad_bf[d * 64:(d + 1) * 64, :W],
                               xpad_bf[d * 64:(d + 1) * 64, 1:W + 1]],
                              axis=0).T           # [W, 128] row e
        full = np.zeros((WROWS, 128), bf16)
        full[:W] = rows
        xT[d] = full.reshape(WROWS // 128, 128, 128).transpose(1, 0, 2) \
                    .reshape(128, WROWS)
    xP = np.ascontiguousarray(xpad_bf[:, :W].reshape(2, 128, W))

    wconv = np.zeros((2, 3, 128, 44), bf16)
    for cb in range(2):
        for k in range(3):
            wconv[cb, k, :, 0:12] = w_off[:, cb * 128:(cb + 1) * 128, k].T
            wconv[cb, k, :, 32:44] = w_mask[:, cb * 128:(cb + 1) * 128, k].T
    iotas = np.zeros((2, 128, CH), np.float32)
    col = np.arange(CH, dtype=np.float32)
    for t in range(2):
        for cb in range(4):
            for r in range(12):
                iotas[t, 32 * cb + r, :] = 512 * (4 * t + cb) + col + (r % 3) - 1 + HALO
    boff_c = b_off.astype(np.float32).reshape(12, 1)
    bmask_c = b_mask.astype(np.float32).reshape(12, 1)

    wmain = np.zeros((6, 2, 128, 128), bf16)
    for kb in range(6):
        for half in range(2):
            dk = 2 * kb + half
            d, k = dk // 3, dk % 3
            wblock = weight[:, d * 64:(d + 1) * 64, k]
            for mb in range(2):
                wmain[kb, mb, 64 * half:64 * half + 64, :] = \
                    wblock[mb * 128:(mb + 1) * 128, :].T
    bmain = bias.astype(np.float32).reshape(2, 128, 1)

    wsel = np.zeros((12, 24, 128), bf16)
    for dk in range(12):
        wsel[dk, dk, 0:64] = 1.0
        wsel[dk, 12 + dk, 64:128] = 1.0
    return {"xT": xT, "xP": xP, "wconv": wconv, "iotas": iotas,
            "boff": boff_c, "bmask": bmask_c,
            "wmain": wmain, "bmain": bmain, "wsel": wsel}


_CACHED = {}


def kernel(x, w_off, b_off, w_mask, b_mask, weight, bias):
    x = np.asarray(x, np.float32)
    w_off = np.asarray(w_off, np.float32)
    b_off = np.asarray(b_off, np.float32)
    w_mask = np.asarray(w_mask, np.float32)
    b_mask = np.asarray(b_mask, np.float32)
    weight = np.asarray(weight, np.float32)
    bias = np.asarray(bias, np.float32)

    if "nc" not in _CACHED:
        _CACHED["nc"] = build_program(1)
    nc = _CACHED["nc"]

    in_maps = [
        _prep_core_inputs(x, w_off, b_off, w_mask, b_mask, weight, bias,
                          core // 2, core % 2)
        for core in range(N_CORES)
    ]
    from concourse.bass_utils import run_bass_kernel_spmd
    res = run_bass_kernel_spmd(nc, in_maps, core_ids=list(range(N_CORES)))
    out = np.zeros((B, C, L), np.float32)
    for core in range(N_CORES):
        b, h = core // 2, core % 2
        y = res.results[core]["y"]
        out[b, 0:128, h * LH:(h + 1) * LH] = y[0]
        out[b, 128:256, h * LH:(h + 1) * LH] = y[1]
    return out



# revision 3
# speedup vs baseline: 1.0186x; 1.0186x over previous
"""Deformable Conv1d (B=4, C=256, L=8192, K=3, DG=4) on 8 Trainium2 cores, v2.

Sharding: core = (sample b = core//2, L-half h = core%2); each core computes
out[b, :, h*4096:(h+1)*4096].

Column order ("j-order"): all post-gather tensors use gather-native column
order j where l(j) = 2048*t + 512*bb + 256*hh + m, with p = j%16 = 4*bb+2*t+hh
and m = j//16.  Conv/chain run in plain l-order split into 8 chunks
(chunk = 4*t + bb, rows 32*bb + (3*d+k)).  The scramble is absorbed by
strided APs at the A-plane writes and the output drain.

Per-core pipeline:
  conv (PE): offset/mask convs, 6 shifted-rhs matmuls/chunk -> psum[64,512]
    (off rows 0-11, mask rows 32-43), ACT drains (Identity+bias / Sigmoid+bias).
  chain (DVE, [128,512] tiles, rows 32b+j): p = pk+iota; e16 = floor(p)
    (ts sub 0.5 -> int16); clamp; p0 = copy(e16); t = p-p0; u = 1-t;
    a1 = t*mk -> A[32:44] (j-order cols); a0 = u*mk -> A[0:12].
  idx spread: 16 pack DMAs -> PACK3[48,1024], 8 replica DMAs -> IDXT[128,3072].
  gather: 96 x 512-idx SWDGE transpose-gathers from per-dgroup pair tables
    (row e = [x[c,e] for c in dgroup] ++ [x[c,e+1]]), out cols in j-order.
  A-broadcast (PE): wsel[48,128] one-hot selector matmul -> at[128,512] per
    (dk, chunk); ACT drain to bf16; modulate in-place: gt *= at (DVE).
  main (PE): 1536-row contraction (12 dk-blocks of [128] = v0|v1 halves,
    pair-add folded into the contraction); ACT drain w/ bias + col-descramble;
    output bf16, upcast on host.
"""
import sys
sys.path.insert(0, '/opt/trn_rl_repo')
from contextlib import ExitStack
import numpy as np
import ml_dtypes

import concourse.bass as bass
import concourse.tile as tile
from concourse import bacc, mybir

dt = mybir.dt
bf16 = ml_dtypes.bfloat16

B, C, L = 4, 256, 8192
N_CORES = 8
LH = L // 2
WX = 4104                  # xP width: cols [q0-1, q0+4097) padded
WROWS = 33 * 128           # 4224 pair-table rows; row e = pos q0 + e - 4
AF = mybir.ActivationFunctionType
ALU = mybir.AluOpType


def build_program(n_reps=1, skip=()):
    skip = set(skip)
    nc = bacc.Bacc("TRN2", target_bir_lowering=False, debug=False,
                   enable_asserts=True, num_devices=N_CORES,
                   num_swdge_queues=2, dynamic_dma_scratch_size=24576)

    def din(name, shape, dty):
        return nc.dram_tensor(name, shape, dty, kind="ExternalInput").ap()

    xT = din("xT", (4, WROWS, 128), dt.bfloat16)
    xP = din("xP", (2, 128, WX), dt.bfloat16)
    wconv = din("wconv", (2, 3, 128, 64), dt.bfloat16)
    iotas = din("iotas", (2, 128, 512), dt.float32)
    boff = din("boff", (12, 1), dt.float32)
    bmask = din("bmask", (12, 1), dt.float32)
    wmain = din("wmain", (12, 2, 128, 128), dt.bfloat16)
    bmain = din("bmain", (2, 128, 1), dt.float32)
    wsel = din("wsel", (12, 48, 128), dt.bfloat16)
    yout = nc.dram_tensor("y", (2, 128, LH), dt.bfloat16,
                          kind="ExternalOutput").ap()

    with ExitStack() as ctx:
        tc = ctx.enter_context(tile.TileContext(nc))
        cpool = ctx.enter_context(tc.tile_pool(name="const", bufs=1))
        ppool = ctx.enter_context(tc.tile_pool(name="planes", bufs=1))
        atpool = ctx.enter_context(tc.tile_pool(name="at", bufs=4))
        psc = ctx.enter_context(tc.tile_pool(name="psc", bufs=2, space="PSUM"))
        psb = ctx.enter_context(tc.tile_pool(name="psb", bufs=2, space="PSUM"))
        psm = ctx.enter_context(tc.tile_pool(name="psm", bufs=3, space="PSUM"))

        t_xP = [cpool.tile([128, WX], dt.bfloat16, tag=f"xP{cb}", name=f"xP{cb}")
                for cb in range(2)]
        for cb in range(2):
            nc.sync.dma_start(t_xP[cb][:], xP[cb])
        t_wconv = [[cpool.tile([128, 64], dt.bfloat16, tag=f"wc{cb}{k}",
                               name=f"wc{cb}{k}") for k in range(3)]
                   for cb in range(2)]
        for cb in range(2):
            for k in range(3):
                nc.sync.dma_start(t_wconv[cb][k][:], wconv[cb, k])
        t_iot = [cpool.tile([128, 512], dt.float32, tag=f"iot{t}", name=f"iot{t}")
                 for t in range(2)]
        for t in range(2):
            nc.sync.dma_start(t_iot[t][:], iotas[t])
        t_boff = cpool.tile([12, 1], dt.float32, name="boff")
        nc.sync.dma_start(t_boff[:], boff[:])
        t_bmask = cpool.tile([12, 1], dt.float32, name="bmask")
        nc.sync.dma_start(t_bmask[:], bmask[:])
        t_wmain = [[cpool.tile([128, 128], dt.bfloat16, tag=f"wm{dk}{mb}",
                               name=f"wm{dk}{mb}") for mb in range(2)]
                   for dk in range(12)]
        for dk in range(12):
            for mb in range(2):
                nc.sync.dma_start(t_wmain[dk][mb][:], wmain[dk, mb])
        t_bmain = [cpool.tile([128, 1], dt.float32, tag=f"bm{mb}", name=f"bm{mb}")
                   for mb in range(2)]
        for mb in range(2):
            nc.sync.dma_start(t_bmain[mb][:], bmain[mb])
        t_wsel = [cpool.tile([48, 128], dt.bfloat16, tag=f"sel{dk}",
                             name=f"sel{dk}") for dk in range(12)]
        for dk in range(12):
            nc.sync.dma_start(t_wsel[dk][:], wsel[dk])

        # persistent chain tiles (memset once: garbage rows stay finite)
        t_pk = [ppool.tile([128, 512], dt.float32, tag=f"pk{t}", name=f"pk{t}")
                for t in range(2)]
        t_mk = [ppool.tile([128, 512], dt.float32, tag=f"mk{t}", name=f"mk{t}")
                for t in range(2)]
        t_pu = [ppool.tile([128, 512], dt.float32, tag=f"pu{t}", name=f"pu{t}")
                for t in range(2)]
        t_e16 = [ppool.tile([128, 512], dt.int16, tag=f"e16{t}", name=f"e16{t}")
                 for t in range(2)]
        t_A = ppool.tile([48, 4096], dt.bfloat16, tag="A", name="A")
        t_pack = ppool.tile([48, 1024], dt.int16, tag="pack", name="pack")
        t_idxt = ppool.tile([128, 3072], dt.int16, tag="idxt", name="idxt")
        t_gt = [ppool.tile([128, 4096], dt.bfloat16, tag=f"gt{dk}",
                           name=f"gt{dk}") for dk in range(12)]
        t_ot = [ppool.tile([128, 4096], dt.bfloat16, tag=f"ot{mb}",
                           name=f"ot{mb}") for mb in range(2)]
        for t in range(2):
            nc.vector.memset(t_pk[t][:], 0.0)
            nc.vector.memset(t_mk[t][:], 0.0)
        nc.vector.memset(t_A[:], 0.0)

        for _rep in range(n_reps):
            # ---- conv ---------------------------------------------------
            for c in range(8 if "conv" not in skip else 0):
                t, b = c // 4, c % 4
                ps = psc.tile([64, 512], dt.float32, tag="convps", name="convps")
                for cb in range(2):
                    for k in range(3):
                        rhs = t_xP[cb][:, 512 * c + k: 512 * c + k + 512]
                        nc.tensor.matmul(ps[:], t_wconv[cb][k][:], rhs,
                                         start=(cb == 0 and k == 0),
                                         stop=(cb == 1 and k == 2))
                nc.scalar.activation(t_pk[t][32 * b:32 * b + 12, :], ps[0:12, :],
                                     AF.Identity, bias=t_boff[:], scale=1.0)
                nc.scalar.activation(t_mk[t][32 * b:32 * b + 12, :], ps[32:44, :],
                                     AF.Sigmoid, bias=t_bmask[:], scale=1.0)

            # ---- chain --------------------------------------------------
            for t in range(2 if "chain" not in skip else 0):
                nc.vector.tensor_add(t_pk[t][:], t_pk[t][:], t_iot[t][:])
                nc.vector.tensor_scalar(t_e16[t][:], t_pk[t][:], 0.5, None,
                                        ALU.subtract)
                nc.vector.tensor_scalar_max(t_e16[t][:], t_e16[t][:], 0)
                nc.vector.tensor_scalar_min(t_e16[t][:], t_e16[t][:], WROWS - 1)
                nc.vector.tensor_copy(t_pu[t][:], t_e16[t][:])
                nc.vector.tensor_sub(t_pk[t][:], t_pk[t][:], t_pu[t][:])  # = frac
                nc.vector.tensor_scalar(t_pu[t][:], t_pk[t][:], -1.0, 1.0,
                                        ALU.mult, ALU.add)                # = 1-frac
                # A writes: cols j = 16*m + 4*b + 2*t + h
                for b in range(4):
                    a1v = t_A[32:44, :].rearrange("a (m q) -> a q m", q=16) \
                        [:, 4 * b + 2 * t: 4 * b + 2 * t + 2, :]
                    nc.vector.tensor_mul(a1v, t_pk[t][32 * b:32 * b + 12, :]
                                         .rearrange("a (h m) -> a h m", h=2),
                                         t_mk[t][32 * b:32 * b + 12, :]
                                         .rearrange("a (h m) -> a h m", h=2))
                    a0v = t_A[0:12, :].rearrange("a (m q) -> a q m", q=16) \
                        [:, 4 * b + 2 * t: 4 * b + 2 * t + 2, :]
                    nc.vector.tensor_mul(a0v, t_pu[t][32 * b:32 * b + 12, :]
                                         .rearrange("a (h m) -> a h m", h=2),
                                         t_mk[t][32 * b:32 * b + 12, :]
                                         .rearrange("a (h m) -> a h m", h=2))

            # ---- idx spread --------------------------------------------
            # stage A: PACK3[3p+rr, 256*dd+m] = e16(dk=4rr+dd, l(p,m))
            for b in range(4 if "spread" not in skip else 0):
                for t in range(2):
                    for h in range(2):
                        p = 4 * b + 2 * t + h
                        eng = nc.sync if (p % 2 == 0) else nc.scalar
                        eng.dma_start(
                            t_pack[3 * p:3 * p + 3, :],
                            t_e16[t][32 * b:32 * b + 12,
                                     256 * h:256 * h + 256])
            # stage B: replicate to 8 Q7-core stripes
            for g in range(8 if "spread" not in skip else 0):
                eng = nc.sync if (g % 2 == 0) else nc.scalar
                eng.dma_start(t_idxt[16 * g:16 * g + 16, :], t_pack[:])

            # ---- gather / A-broadcast / modulate / main, chunk-major ----
            do_g = "gather" not in skip
            do_a = "absel" not in skip
            do_m = "main" not in skip
            for c in range(8):
                for dk in range(12):
                    if do_g:
                        nc.gpsimd.dma_gather(
                            t_gt[dk][:, 512 * c:512 * c + 512].unsqueeze(1),
                            xT[dk // 3],
                            t_idxt[:, 256 * dk + 32 * c: 256 * dk + 32 * c + 32],
                            num_idxs=512, num_idxs_reg=512, elem_size=128,
                            transpose=True, queue_num=(c * 12 + dk) % 2)
                    if do_a:
                        bps = psb.tile([128, 512], dt.float32, tag="bcps",
                                       name="bcps")
                        nc.tensor.matmul(bps[:], t_wsel[dk][:],
                                         t_A[:, 512 * c:512 * c + 512],
                                         start=True, stop=True)
                        at = atpool.tile([128, 512], dt.bfloat16, tag="at",
                                         name="at")
                        nc.scalar.copy(at[:], bps[:])
                        nc.vector.tensor_mul(t_gt[dk][:, 512 * c:512 * c + 512],
                                             t_gt[dk][:, 512 * c:512 * c + 512],
                                             at[:])
                for mb in range(2 if do_m else 0):
                    mps = psm.tile([128, 512], dt.float32, tag="mps",
                                   name="mps")
                    for dk in range(12):
                        nc.tensor.matmul(mps[:], t_wmain[dk][mb][:],
                                         t_gt[dk][:, 512 * c:512 * c + 512],
                                         start=(dk == 0), stop=(dk == 11))
                    for t in range(2):
                        oview = t_ot[mb][:].rearrange(
                            "a (t bb h m) -> a t m bb h",
                            t=2, bb=4, h=2, m=256)[:, t, 32 * c:32 * c + 32]
                        iview = mps[:].rearrange(
                            "a (m bb t h) -> a t m bb h",
                            m=32, bb=4, t=2, h=2)[:, t]
                        nc.scalar.activation(oview, iview, AF.Identity,
                                             bias=t_bmain[mb][:], scale=1.0)
            for mb in range(2):
                nc.sync.dma_start(yout[mb], t_ot[mb][:])

    nc.compile()
    return nc


# ---------------------------------------------------------------------------

def _prep_core_inputs(x, w_off, b_off, w_mask, b_mask, weight, bias, b, h):
    q0 = h * LH

    # xP: conv rhs, col m of xP = x[:, q0 - 1 + m], zero-padded
    xpad = np.zeros((C, WX), np.float32)
    lo, hi = max(0, q0 - 1), min(L, q0 - 1 + WX)
    xpad[:, lo - (q0 - 1):hi - (q0 - 1)] = x[b][:, lo:hi]
    xP = np.ascontiguousarray(xpad.astype(bf16).reshape(2, 128, WX))

    # xT pair tables (HBM rows): row e = [x[c, g], x[c, g+1]], g = q0 + e - 4
    xg = np.zeros((C, WROWS + 1), np.float32)
    g0 = q0 - 4
    lo, hi = max(0, g0), min(L, g0 + WROWS + 1)
    xg[:, lo - g0:hi - g0] = x[b][:, lo:hi]
    xg_bf = xg.astype(bf16)
    xT = np.zeros((4, WROWS, 128), bf16)
    for d in range(4):
        xT[d, :, 0:64] = xg_bf[d * 64:(d + 1) * 64, :WROWS].T
        xT[d, :, 64:128] = xg_bf[d * 64:(d + 1) * 64, 1:WROWS + 1].T

    wconv = np.zeros((2, 3, 128, 64), bf16)
    for cb in range(2):
        for k in range(3):
            wconv[cb, k, :, 0:12] = w_off[:, cb * 128:(cb + 1) * 128, k].T
            wconv[cb, k, :, 32:44] = w_mask[:, cb * 128:(cb + 1) * 128, k].T

    # iota: p_b = off + l + k + 3 (biased table index);  rows 32*b + (3d+k)
    iotas = np.zeros((2, 128, 512), np.float32)
    col = np.arange(512, dtype=np.float32)
    for t in range(2):
        for b in range(4):
            for j in range(12):
                iotas[t, 32 * b + j, :] = 512 * (4 * t + b) + col + (j % 3) + 3

    wmain = np.zeros((12, 2, 128, 128), bf16)
    for dk in range(12):
        d, k = dk // 3, dk % 3
        wblock = weight[:, d * 64:(d + 1) * 64, k]      # [256, 64]
        for mb in range(2):
            wmain[dk, mb, 0:64, :] = wblock[mb * 128:(mb + 1) * 128, :].T
            wmain[dk, mb, 64:128, :] = wblock[mb * 128:(mb + 1) * 128, :].T

    wsel = np.zeros((12, 48, 128), bf16)
    for dk in range(12):
        wsel[dk, dk, 0:64] = 1.0
        wsel[dk, 32 + dk, 64:128] = 1.0

    return {"xT": xT, "xP": xP, "wconv": wconv, "iotas": iotas,
            "boff": b_off.astype(np.float32).reshape(12, 1),
            "bmask": b_mask.astype(np.float32).reshape(12, 1),
            "wmain": wmain,
            "bmain": bias.astype(np.float32).reshape(2, 128, 1),
            "wsel": wsel}


_CACHED = {}


def kernel(x, w_off, b_off, w_mask, b_mask, weight, bias):
    x = np.asarray(x, np.float32)
    w_off = np.asarray(w_off, np.float32)
    b_off = np.asarray(b_off, np.float32)
    w_mask = np.asarray(w_mask, np.float32)
    b_mask = np.asarray(b_mask, np.float32)
    weight = np.asarray(weight, np.float32)
    bias = np.asarray(bias, np.float32)

    if "nc" not in _CACHED:
        _CACHED["nc"] = build_program(1)
    nc = _CACHED["nc"]

    in_maps = [
        _prep_core_inputs(x, w_off, b_off, w_mask, b_mask, weight, bias,
                          core // 2, core % 2)
        for core in range(N_CORES)
    ]
    from concourse.bass_utils import run_bass_kernel_spmd
    res = run_bass_kernel_spmd(nc, in_maps, core_ids=list(range(N_CORES)))
    out = np.zeros((B, C, L), np.float32)
    for core in range(N_CORES):
        b, h = core // 2, core % 2
        y = res.results[core]["y"].astype(np.float32)
        out[b, 0:128, h * LH:(h + 1) * LH] = y[0]
        out[b, 128:256, h * LH:(h + 1) * LH] = y[1]
    return out


# revision 4
# speedup vs baseline: 1.6730x; 1.6426x over previous
"""Deformable Conv1d (B=4, C=256, L=8192, K=3, DG=4) on 8 Trainium2 cores, v2.

Sharding: core = (sample b = core//2, L-half h = core%2); each core computes
out[b, :, h*4096:(h+1)*4096].

Column order ("j-order"): all post-gather tensors use gather-native column
order j where l(j) = 2048*t + 512*bb + 256*hh + m, with p = j%16 = 4*bb+2*t+hh
and m = j//16.  Conv/chain run in plain l-order split into 8 chunks
(chunk = 4*t + bb, rows 32*bb + (3*d+k)).  The scramble is absorbed by
strided APs at the A-plane writes and the output drain.

Per-core pipeline:
  conv (PE): offset/mask convs, 6 shifted-rhs matmuls/chunk -> psum[64,512]
    (off rows 0-11, mask rows 32-43), ACT drains (Identity+bias / Sigmoid+bias).
  chain (DVE, [128,512] tiles, rows 32b+j): p = pk+iota; e16 = floor(p)
    (ts sub 0.5 -> int16); clamp; p0 = copy(e16); t = p-p0; u = 1-t;
    a1 = t*mk -> A[32:44] (j-order cols); a0 = u*mk -> A[0:12].
  idx spread: 16 pack DMAs -> PACK3[48,1024], 8 replica DMAs -> IDXT[128,3072].
  gather: 96 x 512-idx SWDGE transpose-gathers from per-dgroup pair tables
    (row e = [x[c,e] for c in dgroup] ++ [x[c,e+1]]), out cols in j-order.
  A-broadcast (PE): wsel[48,128] one-hot selector matmul -> at[128,512] per
    (dk, chunk); ACT drain to bf16; modulate in-place: gt *= at (DVE).
  main (PE): 1536-row contraction (12 dk-blocks of [128] = v0|v1 halves,
    pair-add folded into the contraction); ACT drain w/ bias + col-descramble;
    output bf16, upcast on host.
"""
import sys
sys.path.insert(0, '/opt/trn_rl_repo')
from contextlib import ExitStack
import numpy as np
import ml_dtypes

import concourse.bass as bass
import concourse.tile as tile
from concourse import bacc, mybir

dt = mybir.dt
bf16 = ml_dtypes.bfloat16

B, C, L = 4, 256, 8192
N_CORES = 8
LH = L // 2
WX = 4104                  # xP width: cols [q0-1, q0+4097) padded
WROWS = 33 * 128           # 4224 pair-table rows; row e = pos q0 + e - 4
AF = mybir.ActivationFunctionType
ALU = mybir.AluOpType


def build_program(n_reps=1, skip=()):
    skip = set(skip)
    nc = bacc.Bacc("TRN2", target_bir_lowering=False, debug=False,
                   enable_asserts=True, num_devices=N_CORES,
                   num_swdge_queues=2, dynamic_dma_scratch_size=24576)

    def din(name, shape, dty):
        return nc.dram_tensor(name, shape, dty, kind="ExternalInput").ap()

    xT = din("xT", (4, WROWS, 128), dt.bfloat16)
    xP = din("xP", (2, 128, WX), dt.bfloat16)
    wconv = din("wconv", (2, 3, 128, 64), dt.bfloat16)
    iotas = din("iotas", (2, 128, 512), dt.float32)
    boff = din("boff", (12, 1), dt.float32)
    bmask = din("bmask", (12, 1), dt.float32)
    wmain = din("wmain", (12, 2, 128, 128), dt.bfloat16)
    bmain = din("bmain", (2, 128, 1), dt.float32)
    wsel = din("wsel", (12, 48, 128), dt.bfloat16)
    yout = nc.dram_tensor("y", (2, 128, LH), dt.bfloat16,
                          kind="ExternalOutput").ap()

    with ExitStack() as ctx:
        tc = ctx.enter_context(tile.TileContext(nc))
        cpool = ctx.enter_context(tc.tile_pool(name="const", bufs=1))
        ppool = ctx.enter_context(tc.tile_pool(name="planes", bufs=1))
        atpool = ctx.enter_context(tc.tile_pool(name="at", bufs=4))
        psc = ctx.enter_context(tc.tile_pool(name="psc", bufs=2, space="PSUM"))
        psb = ctx.enter_context(tc.tile_pool(name="psb", bufs=2, space="PSUM"))
        psm = ctx.enter_context(tc.tile_pool(name="psm", bufs=4, space="PSUM"))

        t_xP = [cpool.tile([128, WX], dt.bfloat16, tag=f"xP{cb}", name=f"xP{cb}")
                for cb in range(2)]
        for cb in range(2):
            nc.sync.dma_start(t_xP[cb][:], xP[cb])
        t_wconv = [[cpool.tile([128, 64], dt.bfloat16, tag=f"wc{cb}{k}",
                               name=f"wc{cb}{k}") for k in range(3)]
                   for cb in range(2)]
        for cb in range(2):
            for k in range(3):
                nc.sync.dma_start(t_wconv[cb][k][:], wconv[cb, k])
        t_iot = [cpool.tile([128, 512], dt.float32, tag=f"iot{t}", name=f"iot{t}")
                 for t in range(2)]
        for t in range(2):
            nc.sync.dma_start(t_iot[t][:], iotas[t])
        t_boff = cpool.tile([12, 1], dt.float32, name="boff")
        nc.sync.dma_start(t_boff[:], boff[:])
        t_bmask = cpool.tile([12, 1], dt.float32, name="bmask")
        nc.sync.dma_start(t_bmask[:], bmask[:])
        t_wmain = [[cpool.tile([128, 128], dt.bfloat16, tag=f"wm{dk}{mb}",
                               name=f"wm{dk}{mb}") for mb in range(2)]
                   for dk in range(12)]
        for dk in range(12):
            for mb in range(2):
                nc.sync.dma_start(t_wmain[dk][mb][:], wmain[dk, mb])
        t_bmain = [cpool.tile([128, 1], dt.float32, tag=f"bm{mb}", name=f"bm{mb}")
                   for mb in range(2)]
        for mb in range(2):
            nc.sync.dma_start(t_bmain[mb][:], bmain[mb])
        t_wsel = [cpool.tile([48, 128], dt.bfloat16, tag=f"sel{dk}",
                             name=f"sel{dk}") for dk in range(12)]
        for dk in range(12):
            nc.sync.dma_start(t_wsel[dk][:], wsel[dk])

        # persistent chain tiles (memset once: garbage rows stay finite)
        t_pk = [ppool.tile([128, 512], dt.float32, tag=f"pk{t}", name=f"pk{t}")
                for t in range(2)]
        t_mk = [ppool.tile([128, 512], dt.float32, tag=f"mk{t}", name=f"mk{t}")
                for t in range(2)]
        t_pu = [ppool.tile([128, 512], dt.float32, tag=f"pu{t}", name=f"pu{t}")
                for t in range(2)]
        t_e16 = [ppool.tile([128, 512], dt.int16, tag=f"e16{t}", name=f"e16{t}")
                 for t in range(2)]
        t_A = ppool.tile([48, 4096], dt.bfloat16, tag="A", name="A")
        t_pack = ppool.tile([48, 1024], dt.int16, tag="pack", name="pack")
        t_idxt = ppool.tile([128, 3072], dt.int16, tag="idxt", name="idxt")
        t_gt = [ppool.tile([128, 4096], dt.bfloat16, tag=f"gt{dk}",
                           name=f"gt{dk}") for dk in range(12)]
        t_ot = [ppool.tile([128, 4096], dt.bfloat16, tag=f"ot{mb}",
                           name=f"ot{mb}") for mb in range(2)]
        for t in range(2):
            nc.vector.memset(t_pk[t][:], 0.0)
            nc.vector.memset(t_mk[t][:], 0.0)
        nc.vector.memset(t_A[:], 0.0)

        for _rep in range(n_reps):
            # ---- conv ---------------------------------------------------
            for c in range(8 if "conv" not in skip else 0):
                t, b = c // 4, c % 4
                ps = psc.tile([64, 512], dt.float32, tag="convps", name="convps")
                for cb in range(2):
                    for k in range(3):
                        rhs = t_xP[cb][:, 512 * c + k: 512 * c + k + 512]
                        nc.tensor.matmul(ps[:], t_wconv[cb][k][:], rhs,
                                         start=(cb == 0 and k == 0),
                                         stop=(cb == 1 and k == 2))
                nc.scalar.activation(t_pk[t][32 * b:32 * b + 12, :], ps[0:12, :],
                                     AF.Identity, bias=t_boff[:], scale=1.0)
                nc.scalar.activation(t_mk[t][32 * b:32 * b + 12, :], ps[32:44, :],
                                     AF.Sigmoid, bias=t_bmask[:], scale=1.0)

            # ---- chain part 1: indices ---------------------------------
            for t in range(2 if "chain" not in skip else 0):
                nc.vector.tensor_add(t_pk[t][:], t_pk[t][:], t_iot[t][:])
                nc.vector.tensor_scalar(t_e16[t][:], t_pk[t][:], 0.5, None,
                                        ALU.subtract)
                nc.vector.tensor_scalar_max(t_e16[t][:], t_e16[t][:], 0)
                nc.vector.tensor_scalar_min(t_e16[t][:], t_e16[t][:], WROWS - 1)

            # ---- idx spread (moved before interp/A so gathers start early)
            for b in range(4 if "spread" not in skip else 0):
                for t in range(2):
                    for h in range(2):
                        p = 4 * b + 2 * t + h
                        eng = nc.sync if (p % 2 == 0) else nc.scalar
                        eng.dma_start(
                            t_pack[3 * p:3 * p + 3, :],
                            t_e16[t][32 * b:32 * b + 12,
                                     256 * h:256 * h + 256])
            for g in range(8 if "spread" not in skip else 0):
                eng = nc.sync if (g % 2 == 0) else nc.scalar
                eng.dma_start(t_idxt[16 * g:16 * g + 16, :], t_pack[:])

            # ---- chain part 2: interp weights --------------------------
            for t in range(2 if "chain" not in skip else 0):
                nc.vector.tensor_copy(t_pu[t][:], t_e16[t][:])
                nc.vector.tensor_sub(t_pk[t][:], t_pk[t][:], t_pu[t][:])  # = frac
                nc.vector.tensor_scalar(t_pu[t][:], t_pk[t][:], -1.0, 1.0,
                                        ALU.mult, ALU.add)                # = 1-frac
                # A writes: cols j = 16*m + 4*b + 2*t + h
                for b in range(4):
                    a1v = t_A[32:44, :].rearrange("a (m q) -> a q m", q=16) \
                        [:, 4 * b + 2 * t: 4 * b + 2 * t + 2, :]
                    nc.vector.tensor_mul(a1v, t_pk[t][32 * b:32 * b + 12, :]
                                         .rearrange("a (h m) -> a h m", h=2),
                                         t_mk[t][32 * b:32 * b + 12, :]
                                         .rearrange("a (h m) -> a h m", h=2))
                    a0v = t_A[0:12, :].rearrange("a (m q) -> a q m", q=16) \
                        [:, 4 * b + 2 * t: 4 * b + 2 * t + 2, :]
                    nc.vector.tensor_mul(a0v, t_pu[t][32 * b:32 * b + 12, :]
                                         .rearrange("a (h m) -> a h m", h=2),
                                         t_mk[t][32 * b:32 * b + 12, :]
                                         .rearrange("a (h m) -> a h m", h=2))

            # ---- gather / A-broadcast / modulate / main, chunk-major ----
            do_g = "gather" not in skip
            do_a = "absel" not in skip
            do_m = "main" not in skip
            for c in range(8):
                for dk in range(12):
                    if do_g:
                        nc.gpsimd.dma_gather(
                            t_gt[dk][:, 512 * c:512 * c + 512].unsqueeze(1),
                            xT[dk // 3],
                            t_idxt[:, 256 * dk + 32 * c: 256 * dk + 32 * c + 32],
                            num_idxs=512, num_idxs_reg=512, elem_size=128,
                            transpose=True, queue_num=(c * 12 + dk) % 2)
                    if do_a:
                        bps = psb.tile([128, 512], dt.float32, tag="bcps",
                                       name="bcps")
                        nc.tensor.matmul(bps[:], t_wsel[dk][:],
                                         t_A[:, 512 * c:512 * c + 512],
                                         start=True, stop=True)
                        at = atpool.tile([128, 512], dt.bfloat16, tag="at",
                                         name="at")
                        nc.scalar.copy(at[:], bps[:])
                        nc.vector.tensor_mul(t_gt[dk][:, 512 * c:512 * c + 512],
                                             t_gt[dk][:, 512 * c:512 * c + 512],
                                             at[:])
                for mb in range(2 if do_m else 0):
                    mps = psm.tile([128, 512], dt.float32, tag="mps",
                                   name="mps")
                    for dk in range(12):
                        nc.tensor.matmul(mps[:], t_wmain[dk][mb][:],
                                         t_gt[dk][:, 512 * c:512 * c + 512],
                                         start=(dk == 0), stop=(dk == 11))
                    for t in range(2):
                        oview = t_ot[mb][:].rearrange(
                            "a (t bb h m) -> a t m bb h",
                            t=2, bb=4, h=2, m=256)[:, t, 32 * c:32 * c + 32]
                        iview = mps[:].rearrange(
                            "a (m bb t h) -> a t m bb h",
                            m=32, bb=4, t=2, h=2)[:, t]
                        nc.scalar.activation(oview, iview, AF.Identity,
                                             bias=t_bmain[mb][:], scale=1.0)
            for mb in range(2):
                nc.sync.dma_start(yout[mb], t_ot[mb][:])

    nc.compile()
    return nc


# ---------------------------------------------------------------------------

def _prep_core_inputs(x, w_off, b_off, w_mask, b_mask, weight, bias, b, h):
    q0 = h * LH

    # xP: conv rhs, col m of xP = x[:, q0 - 1 + m], zero-padded
    xpad = np.zeros((C, WX), np.float32)
    lo, hi = max(0, q0 - 1), min(L, q0 - 1 + WX)
    xpad[:, lo - (q0 - 1):hi - (q0 - 1)] = x[b][:, lo:hi]
    xP = np.ascontiguousarray(xpad.astype(bf16).reshape(2, 128, WX))

    # xT pair tables (HBM rows): row e = [x[c, g], x[c, g+1]], g = q0 + e - 4
    xg = np.zeros((C, WROWS + 1), np.float32)
    g0 = q0 - 4
    lo, hi = max(0, g0), min(L, g0 + WROWS + 1)
    xg[:, lo - g0:hi - g0] = x[b][:, lo:hi]
    xg_bf = xg.astype(bf16)
    xT = np.zeros((4, WROWS, 128), bf16)
    for d in range(4):
        xT[d, :, 0:64] = xg_bf[d * 64:(d + 1) * 64, :WROWS].T
        xT[d, :, 64:128] = xg_bf[d * 64:(d + 1) * 64, 1:WROWS + 1].T

    wconv = np.zeros((2, 3, 128, 64), bf16)
    for cb in range(2):
        for k in range(3):
            wconv[cb, k, :, 0:12] = w_off[:, cb * 128:(cb + 1) * 128, k].T
            wconv[cb, k, :, 32:44] = w_mask[:, cb * 128:(cb + 1) * 128, k].T

    # iota: p_b = off + l + k + 3 (biased table index);  rows 32*b + (3d+k)
    iotas = np.zeros((2, 128, 512), np.float32)
    col = np.arange(512, dtype=np.float32)
    for t in range(2):
        for b in range(4):
            for j in range(12):
                iotas[t, 32 * b + j, :] = 512 * (4 * t + b) + col + (j % 3) + 3

    wmain = np.zeros((12, 2, 128, 128), bf16)
    for dk in range(12):
        d, k = dk // 3, dk % 3
        wblock = weight[:, d * 64:(d + 1) * 64, k]      # [256, 64]
        for mb in range(2):
            wmain[dk, mb, 0:64, :] = wblock[mb * 128:(mb + 1) * 128, :].T
            wmain[dk, mb, 64:128, :] = wblock[mb * 128:(mb + 1) * 128, :].T

    wsel = np.zeros((12, 48, 128), bf16)
    for dk in range(12):
        wsel[dk, dk, 0:64] = 1.0
        wsel[dk, 32 + dk, 64:128] = 1.0

    return {"xT": xT, "xP": xP, "wconv": wconv, "iotas": iotas,
            "boff": b_off.astype(np.float32).reshape(12, 1),
            "bmask": b_mask.astype(np.float32).reshape(12, 1),
            "wmain": wmain,
            "bmain": bias.astype(np.float32).reshape(2, 128, 1),
            "wsel": wsel}


_CACHED = {}


def kernel(x, w_off, b_off, w_mask, b_mask, weight, bias):
    x = np.asarray(x, np.float32)
    w_off = np.asarray(w_off, np.float32)
    b_off = np.asarray(b_off, np.float32)
    w_mask = np.asarray(w_mask, np.float32)
    b_mask = np.asarray(b_mask, np.float32)
    weight = np.asarray(weight, np.float32)
    bias = np.asarray(bias, np.float32)

    if "nc" not in _CACHED:
        _CACHED["nc"] = build_program(1)
    nc = _CACHED["nc"]

    in_maps = [
        _prep_core_inputs(x, w_off, b_off, w_mask, b_mask, weight, bias,
                          core // 2, core % 2)
        for core in range(N_CORES)
    ]
    from concourse.bass_utils import run_bass_kernel_spmd
    res = run_bass_kernel_spmd(nc, in_maps, core_ids=list(range(N_CORES)))
    out = np.zeros((B, C, L), np.float32)
    for core in range(N_CORES):
        b, h = core // 2, core % 2
        y = res.results[core]["y"].astype(np.float32)
        out[b, 0:128, h * LH:(h + 1) * LH] = y[0]
        out[b, 128:256, h * LH:(h + 1) * LH] = y[1]
    return out
